# revision 21
# baseline (speedup 1.0000x reference)
"""Trainium2 Bass kernel for nn_Add_PairLinears.

y = sum_a( blockdiag2(W[a]) applied to x[:, perms[a]] ) + sum_a b[a]

Strategy (data-parallel over batch, 8 cores, no collectives):
  - Each core owns a batch shard of 1024 rows. x is pre-cast to bf16 on
    host (precision choice; all layout/compute work stays on device).
  - On device: PE-transpose x to x^T (d on partitions), keeping x^T
    resident in SBUF AND spilling one copy to DRAM (gather source).
  - For each mixer a>0: SWDGE dma_gather pulls the permuted rows from
    the DRAM x^T copy (idx tables sorted by source row for HBM
    locality; the within-tile order is absorbed into lhsT).  Mixer 0
    (identity perm) reads the resident SBUF x^T directly - no DMA.
  - The 2x2 block-diagonal mix exploits PE array packing: the 128x128
    lhsT per (mixer, tile) is block-diagonal, so in 64x64 tiling mode
    each mixer needs only two 64x64 tiles.  Odd mixers' gathered rows
    are placed partition-rotated by 64 so the four tile positions
    (T0/T2/T8/T10) are all used; contributions accumulate into two
    PSUM banks (one per row-half, so same-bank collisions stay on one
    subarray and are HW-serialized), then DVE adds the banks and the
    scalar engine fuses the bias sum_a b[a] while casting to bf16.
  - Output is stored transposed (y^T, bf16) and unsharded/transposed/
    upcast on host.
"""

import numpy as np
import ml_dtypes

import concourse.bass as bass
import concourse.bacc as bacc
import concourse.tile as tile
from concourse import library_config, mybir
from concourse.bass_utils import run_bass_kernel_spmd

B, D, A = 8192, 4096, 8
N_CORES = 8
BC = B // N_CORES          # 1024 batch rows per core
NJ = D // 128              # 32 d-tiles of 128
JG = 4                     # j-tiles per gather group
NG = NJ // JG              # gather groups per mixer
NQ = 4                     # SWDGE queues

F32 = mybir.dt.float32
BF16 = mybir.dt.bfloat16
I16 = mybir.dt.int16

_GRAPH_CACHE = {}
_LAST_RESULTS = None

HB = BC // 2  # batch-half width (512)


def _build_graph():
    nc = bacc.Bacc(None, num_swdge_queues=NQ)

    x_ext = nc.declare_dram_parameter("x", [BC, D], BF16, isOutput=False)
    lhsT_ext = nc.declare_dram_parameter("lhsT", [NJ, 128, A * 64], BF16, isOutput=False)
    idx_ext = nc.declare_dram_parameter("idx", [128, A * 256], I16, isOutput=False)
    bsum_ext = nc.declare_dram_parameter("bsum", [128, NJ], F32, isOutput=False)
    ident_ext = nc.declare_dram_parameter("ident", [128, 128], BF16, isOutput=False)
    yt_ext = nc.declare_dram_parameter("yt", [D, BC], BF16, isOutput=True)

    qn = [0]

    def next_q():
        q = qn[0]
        qn[0] = (q + 1) % NQ
        return q

    with tile.TileContext(nc) as tc:
        with (
            tc.tile_pool(name="const", bufs=1) as constp,
            tc.tile_pool(name="xin", bufs=3) as xinp,
            tc.tile_pool(name="xt", bufs=1) as xtp,
            tc.tile_pool(name="lhs", bufs=1) as lhsp,
            tc.tile_pool(name="g", bufs=18) as gp,
            tc.tile_pool(name="y", bufs=4) as yp,
            tc.tile_pool(name="ps", bufs=8, space="PSUM") as psp,
            tc.tile_pool(name="dram", bufs=1, space="DRAM") as dramp,
        ):
            # small consts via HWDGE so ident lands before the gpsimd
            # library load finishes (transposes need it immediately)
            ident = constp.tile([128, 128], BF16)
            nc.sync.dma_start(out=ident[:], in_=ident_ext[:])
            idx_sb = constp.tile([128, A * 256], I16)
            nc.sync.dma_start(out=idx_sb[:], in_=idx_ext[:])
            bsum_sb = constp.tile([128, NJ], F32)
            nc.sync.dma_start(out=bsum_sb[:], in_=bsum_ext[:])

            nc.gpsimd.load_library(library_config.mlp)

            # all lhsT tiles resident (4.2MB: two 64x64 diag blocks per
            # (mixer, j) instead of a dense 128x128)
            lhs_all = lhsp.tile([128, NJ, A * 64], BF16)
            for j0 in range(0, NJ, 8):
                nc.gpsimd.dma_start(
                    out=lhs_all[:, j0:j0 + 8, :],
                    in_=lhsT_ext[j0:j0 + 8].rearrange("j t m -> t j m"))

            # resident x^T (d on partitions): [128, NJ, BC] bf16
            xt_sb = xtp.tile([128, NJ, BC], BF16)

            yt_v = yt_ext[:].rearrange("(j p) b -> p j b", p=128)

            # per-half x^T DRAM spill (gather source)
            xt_d0 = dramp.tile([D, HB], BF16, tag="xt0")
            xt_d1 = dramp.tile([D, HB], BF16, tag="xt1")
            xt_d = [xt_d0, xt_d1]

            # PE warmup: ramp the p-state while the first x tiles load so
            # the phase-1 transposes run at full clock
            warm = psp.tile([128, 4, 128], BF16, tag="ps")
            for _ in range(24):
                nc.tensor.transpose(warm[:, 0, :], ident[:], ident[:])

            JCH = 8

            def phase1_chunk(h, bt0, jg0):
                """load + PE transpose + evac-to-resident for one x chunk:
                batch tile bt0 of half h, j-tiles [jg0, jg0+JCH)."""
                bt = h * (HB // 128) + bt0
                xtile = xinp.tile([128, JCH * 128], BF16, tag="xin")
                nc.sync.dma_start(
                    out=xtile[:],
                    in_=x_ext[bt * 128:(bt + 1) * 128,
                              jg0 * 128:(jg0 + JCH) * 128])
                for jh in range(JCH // 4):
                    pt = psp.tile([128, 4, 128], BF16, tag="ps")
                    for jq in range(4):
                        jo = jh * 4 + jq
                        nc.tensor.transpose(
                            pt[:, jq, :], xtile[:, jo * 128:(jo + 1) * 128],
                            ident[:])
                    dst = xt_sb[:, jg0 + jh * 4:jg0 + (jh + 1) * 4,
                                bt * 128:(bt + 1) * 128]
                    # alternate PSUM evacuation between DVE and ACT so
                    # neither engine rate-limits phase 1
                    if (bt0 + jh) % 2 == 0:
                        nc.vector.tensor_copy(dst, pt[:])
                    else:
                        nc.scalar.activation(
                            dst, pt[:], mybir.ActivationFunctionType.Copy)

            def spill_rows(h, jg0):
                # spill one JCH-row-block of the half's resident x^T as
                # soon as its transposes land (jg-major phase-1 order), on
                # the scalar HWDGE queue so it doesn't block the x loads
                # pipelining on the sync queue
                xt_dram_v = xt_d[h][:].rearrange("(j p) b -> p j b", p=128)
                nc.scalar.dma_start(
                    out=xt_dram_v[:, jg0:jg0 + JCH, :],
                    in_=xt_sb[:, jg0:jg0 + JCH, h * HB:(h + 1) * HB])

            def mix_group(h, gi):
                """gather + packed mix + store for group gi of batch half h."""
                xt_dram = xt_d[h]
                gts = {}
                for a in range(1, A):
                    gt = gp.tile([128, JG, HB], BF16, tag="g")
                    c0 = a * 256 + gi * (JG * 8)
                    nc.gpsimd.dma_gather(
                        out_ap=gt[:],
                        in_ap=xt_dram[:],
                        idxs_ap=idx_sb[:, c0:c0 + JG * 8],
                        num_idxs=JG * 128,
                        num_idxs_reg=JG * 128,
                        elem_size=HB,
                        queue_num=next_q(),
                    )
                    gts[a] = gt
                # emit all matmuls of the group first (one contiguous PE
                # burst, lets the p-state ramp), then the evacuations
                banks = []
                for jc in range(JG):
                    j = gi * JG + jc
                    pmA = psp.tile([128, 512], F32, tag="ps")
                    pmB = psp.tile([128, 512], F32, tag="ps")
                    banks.append((j, pmA, pmB))

                    def rhs_half(a, ph):
                        if a == 0:
                            return xt_sb[ph * 64:(ph + 1) * 64, j,
                                         h * HB:(h + 1) * HB]
                        return gts[a][ph * 64:(ph + 1) * 64, jc, :]

                    # 64x64 array packing: per grid (pair of mixers) four
                    # tiles T0/T10 (even mixer, natural placement) and
                    # T2/T8 (odd mixer, rows rotated by 64).  Row-half 0
                    # accumulates in pmA, row-half 1 in pmB; same-position
                    # tiles across grids serialize on their subarray.
                    for g in range(A // 2):
                        ae, ao = 2 * g, 2 * g + 1
                        st = (g == 0)
                        sp = (g == A // 2 - 1)
                        nc.tensor.matmul(
                            pmA[0:64, :],
                            lhs_all[0:64, j, ae * 64:(ae + 1) * 64],
                            rhs_half(ae, 0),
                            start=st, stop=sp, tile_position=(0, 0))
                        nc.tensor.matmul(
                            pmB[64:128, :],
                            lhs_all[64:128, j, ae * 64:(ae + 1) * 64],
                            rhs_half(ae, 1),
                            start=st, stop=sp, tile_position=(64, 64))
                        nc.tensor.matmul(
                            pmA[64:128, :],
                            lhs_all[0:64, j, ao * 64:(ao + 1) * 64],
                            rhs_half(ao, 0),
                            start=st, stop=sp, tile_position=(0, 64))
                        nc.tensor.matmul(
                            pmB[0:64, :],
                            lhs_all[64:128, j, ao * 64:(ao + 1) * 64],
                            rhs_half(ao, 1),
                            start=st, stop=sp, tile_position=(64, 0))

                for j, pmA, pmB in banks:
                    # both banks can't feed one DVE op (src0/src1 not both
                    # PSUM): ACT folds bias into bank A -> f32 SBUF, DVE
                    # adds bank B and casts to bf16
                    ytile = yp.tile([128, HB], BF16, tag="y")
                    yA = yp.tile([128, HB], F32, tag="ya")
                    nc.scalar.activation(
                        yA[:],
                        pmA[:],
                        mybir.ActivationFunctionType.Identity,
                        bias=bsum_sb[:, j:j + 1],
                    )
                    nc.vector.tensor_add(ytile[:], pmB[:], yA[:])
                    nc.sync.dma_start(
                        out=yt_v[:, j, h * HB:(h + 1) * HB], in_=ytile[:])

            # phase 1 for half 0 (spilling each batch slice as it lands),
            # then interleave half-1 phase1+spills with the half-0 mix
            # groups so the half-1 gathers can start immediately after
            # jg-major so each JCH row-block of x^T completes early and its
            # spill overlaps the rest of phase 1
            for jg0 in range(0, NJ, JCH):
                for bt0 in range(HB // 128):
                    phase1_chunk(0, bt0, jg0)
                spill_rows(0, jg0)

            p1_chunks = [(bt0, jg0)
                         for jg0 in range(0, NJ, JCH)
                         for bt0 in range(HB // 128)]
            ci = 0
            # front-load half-1 phase1 into the first 4 mix groups so its
            # spills complete well before the half-1 gathers need them
            per_group = (len(p1_chunks) + 3) // 4
            for gi in range(NG):
                mix_group(0, gi)
                for _ in range(per_group):
                    if ci < len(p1_chunks):
                        bt0, jg0 = p1_chunks[ci]
                        phase1_chunk(1, bt0, jg0)
                        ci += 1
                        if bt0 == HB // 128 - 1:
                            spill_rows(1, jg0)
            for gi in range(NG):
                mix_group(1, gi)

    nc.compile()
    return nc


def _host_tables(W, b, perms):
    """Build the device-side constant tables from W/b/perms.

    Per (mixer a, j-tile, output-half hblk) the 64 gather rows are sorted
    by source row (HBM locality) and placed at partition half
    (hblk + a%2) % 2 - odd mixers rotated by 64 so the four 64x64 PE
    tile positions are all used.  lhsT[j, p, a, :] holds the 64x64
    diagonal block row for the gathered row at partition p.
    """
    lhsT = np.zeros((NJ, 128, A, 64), np.float32)
    idx_vals = np.zeros((A, NJ, 128), np.int64)
    Wf = W.reshape(A, D // 2, 2, 2)
    for a in range(A):
        rot = a % 2
        for j in range(NJ):
            for hblk in range(2):
                pos = 128 * j + 64 * hblk + np.arange(64)
                srcs = perms[a, pos].astype(np.int64)
                order = np.argsort(srcs, kind="stable")
                pos_s = pos[order]
                p0 = 64 * ((hblk + rot) % 2)
                idx_vals[a, j, p0:p0 + 64] = srcs[order]
                n = pos_s // 2
                i = pos_s % 2
                o_l = (pos_s - 128 * j - 64 * hblk) & ~1  # local even output
                q = np.arange(64)
                for oo in range(2):
                    lhsT[j, p0 + q, a, o_l + oo] = Wf[a, n, i, oo]

    lhsT = np.ascontiguousarray(
        lhsT.reshape(NJ, 128, A * 64)).astype(ml_dtypes.bfloat16)

    # idx: per mixer, placement-ordered source rows wrapped over 16
    # partitions (index i at [i%16, i//16]), replicated into each Q7
    # core's 16-partition group
    idx = np.zeros((128, A * 256), np.int16)
    for a in range(A):
        w16 = idx_vals[a].reshape(256, 16).astype(np.int16).T
        idx[:, a * 256:(a + 1) * 256] = np.tile(w16, (8, 1))

    bsum = np.ascontiguousarray(
        b.astype(np.float64).sum(axis=0).astype(np.float32).reshape(NJ, 128).T)
    ident = np.eye(128, dtype=np.float32).astype(ml_dtypes.bfloat16)
    return lhsT, idx, bsum, ident


def _host_idx_vals(perms):
    """Placement table (which source row sits at partition p of tile (a,j))."""
    idx_vals = np.zeros((A, NJ, 128), np.int64)
    for a in range(A):
        rot = a % 2
        for j in range(NJ):
            for hblk in range(2):
                pos = 128 * j + 64 * hblk + np.arange(64)
                srcs = np.sort(perms[a, pos].astype(np.int64))
                p0 = 64 * ((hblk + rot) % 2)
                idx_vals[a, j, p0:p0 + 64] = srcs
    return idx_vals


def kernel(x, W, b, perms):
    x = np.asarray(x, dtype=np.float32)
    W = np.asarray(W, dtype=np.float32)
    b = np.asarray(b, dtype=np.float32)
    perms = np.asarray(perms)

    lhsT, idx, bsum, ident = _host_tables(W, b, perms)

    if "nc" not in _GRAPH_CACHE:
        _GRAPH_CACHE["nc"] = _build_graph()
    nc = _GRAPH_CACHE["nc"]

    x_bf = x.astype(ml_dtypes.bfloat16)
    in_maps = []
    for c in range(N_CORES):
        m = {
            "lhsT": lhsT,
            "idx": idx,
            "bsum": bsum,
            "ident": ident,
            "x": np.ascontiguousarray(x_bf[c * BC:(c + 1) * BC]),
        }
        in_maps.append(m)

    res = run_bass_kernel_spmd(nc, in_maps, core_ids=list(range(N_CORES)))
    global _LAST_RESULTS
    _LAST_RESULTS = res
    y = np.concatenate(
        [np.asarray(res.results[c]["yt"], dtype=np.float32).T for c in range(N_CORES)],
        axis=0,
    )
    return np.ascontiguousarray(y)
